# revision 2
# baseline (speedup 1.0000x reference)
"""Trainium2 Bass kernel for nn_NetworksPlusCircuit.

Computation: y[b] = circuit(sigmoid(x[b] @ Ws + bs)) for x [1048576, 64].

Key simplification: the SDD circuit f(i) = pos_i*f(i+1) + neg_i*f(i+2) with
neg = 1-l collapses to f(i) == 1 for all i >= 8 (l + (1-l) = 1), so only
labelling columns 1..7 matter (literals 3 and 7 are categorical). The matmul
shrinks to [B,64] @ [64,7] and the circuit to a handful of elementwise ops:
    f7 = l7 + 1
    f6 = l6*l7 + 1
    f5 = l5*(f6-f7) + f7
    f4 = l4*(f5-f6) + f6
    f3 = l3*f4 + f5
    f2 = l2*(f3-f4) + f4
    f1 = l1*(f2-f3) + f3

Sharding: pure data parallel over batch across 8 cores. Per core the host
pre-transposes its x shard to xT2 [128, 65536] (two 64-row d-major halves
stacked on the partition dim) so the PE streams it as the moving operand
against a small stationary block-diagonal weight W2 [128, 16]. Matmul output
lands literal-on-partition in PSUM; the scalar engine applies sigmoid (with
the bias folded in as a per-partition bias) while copying to SBUF, the DVE
32x32 block-transpose flips batch onto partitions, and the circuit runs as
wide strided elementwise ops. The host inverts the layout permutation on the
gathered outputs.
"""

import sys

for _p in ("/opt/trn_rl_repo",):
    if _p not in sys.path:
        sys.path.insert(0, _p)

import numpy as np

N_CORES = 8
B_TOTAL = 1048576
D = 64
BC = B_TOTAL // N_CORES      # 131072 batch per core
HALF = BC // 2               # 65536 w-columns per half
NS = 8                       # super-iterations per core
SUP_W = HALF // NS           # 8192 w-columns per super-iteration
WCH = 512                    # w-columns per matmul


def _split_multiwait_instructions(nc, mybir):
    """This walrus build accepts at most one sync wait per instruction.
    Split any multi-wait instruction into single-wait NoOps on the same
    engine ahead of it (engines execute their queue in order, so semantics
    are unchanged)."""
    n_split = 0
    for fn in nc.m.functions:
        for blk in fn.blocks:
            insts = blk.instructions
            if not any(
                i.sync_info is not None and len(i.sync_info.on_wait) > 1
                for i in insts
            ):
                continue
            out = []
            for inst in insts:
                si = inst.sync_info
                if si is not None and len(si.on_wait) > 1:
                    waits = list(si.on_wait)
                    for k, w in enumerate(waits[:-1]):
                        nop = mybir.InstNoOp(
                            name=f"{inst.name}-sw{k}",
                            engine=inst.engine,
                            ins=[],
                            outs=[],
                            sync_info=mybir.SyncInfo(on_wait=[w], on_update=[]),
                        )
                        out.append(nop)
                        n_split += 1
                    inst.sync_info = mybir.SyncInfo(
                        on_wait=[waits[-1]], on_update=list(si.on_update)
                    )
                out.append(inst)
            blk.instructions = out
    return n_split


def build_program():
    import concourse.bass as bass
    import concourse.mybir as mybir
    from concourse import tile
    from contextlib import ExitStack

    f32 = mybir.dt.float32
    nc = bass.Bass("TRN2")
    xT2 = nc.dram_tensor("xT2", [128, HALF], f32, kind="ExternalInput")
    w2 = nc.dram_tensor("w2", [128, 16], f32, kind="ExternalInput")
    b2 = nc.dram_tensor("b2", [128, 1], f32, kind="ExternalInput")
    y = nc.dram_tensor("y", [NS, 128, 128], f32, kind="ExternalOutput")

    with tile.TileContext(nc) as tc:
        with ExitStack() as ctx:
            wpool = ctx.enter_context(tc.tile_pool(name="wpool", bufs=1))
            xpool = ctx.enter_context(tc.tile_pool(name="xpool", bufs=2))
            spool = ctx.enter_context(tc.tile_pool(name="spool", bufs=3))
            hpool = ctx.enter_context(tc.tile_pool(name="hpool", bufs=2))
            cpool = ctx.enter_context(tc.tile_pool(name="cpool", bufs=2))
            fpool = ctx.enter_context(tc.tile_pool(name="fpool", bufs=2))
            ppool = ctx.enter_context(
                tc.tile_pool(name="ppool", bufs=4, space="PSUM")
            )

            wt = wpool.tile([128, 16], f32)
            nc.sync.dma_start(wt[:], w2[:, :])
            bt = wpool.tile([128, 1], f32)
            nc.sync.dma_start(bt[:], b2[:, :])

            for s in range(NS):
                X = xpool.tile([128, SUP_W], f32)
                nc.sync.dma_start(X[:], xT2[:, s * SUP_W:(s + 1) * SUP_W])
                H = hpool.tile([128, 2048], f32)
                for u in range(4):
                    ps = ppool.tile([128, WCH], f32)
                    for g in range(4):
                        i = 4 * u + g
                        nc.tensor.matmul(
                            ps[32 * g:32 * g + 16, :],
                            wt[:, :],
                            X[:, WCH * i:WCH * (i + 1)],
                            start=True,
                            stop=True,
                            tile_position=(0, 32 * g),
                        )
                    S = spool.tile([128, WCH], f32)
                    nc.scalar.activation(
                        S[:],
                        ps[:],
                        mybir.ActivationFunctionType.Sigmoid,
                        bias=bt[:, 0:1],
                        scale=1.0,
                    )
                    nc.vector.transpose(H[:, WCH * u:WCH * (u + 1)], S[:])

                # circuit over H [128, 2048]; literal j of (h-half) at free
                # slot 32*blk + 8*h + j
                H3 = H.rearrange("p (b q) -> p b q", q=32)
                l = lambda j: H3[:, :, j:16:8]  # noqa: E731

                def t3(name):
                    t = cpool.tile([128, 128], f32, name=name, tag=name)
                    return t.rearrange("p (b q) -> p b q", q=2)

                F = fpool.tile([128, 128], f32)
                F3 = F.rearrange("p (b q) -> p b q", q=2)

                f7 = t3("f7")
                nc.vector.tensor_scalar_add(f7, l(6), 1.0)
                m6 = t3("m6")
                nc.vector.tensor_mul(m6, l(5), l(6))
                f6 = t3("f6")
                nc.vector.tensor_scalar_add(f6, m6, 1.0)
                d5 = t3("d5")
                nc.vector.tensor_sub(d5, f6, f7)
                p5 = t3("p5")
                nc.vector.tensor_mul(p5, l(4), d5)
                f5 = t3("f5")
                nc.vector.tensor_add(f5, p5, f7)
                d4 = t3("d4")
                nc.vector.tensor_sub(d4, f5, f6)
                p4 = t3("p4")
                nc.vector.tensor_mul(p4, l(3), d4)
                f4 = t3("f4")
                nc.vector.tensor_add(f4, p4, f6)
                p3 = t3("p3")
                nc.vector.tensor_mul(p3, l(2), f4)
                f3 = t3("f3")
                nc.vector.tensor_add(f3, p3, f5)
                d2 = t3("d2")
                nc.vector.tensor_sub(d2, f3, f4)
                p2 = t3("p2")
                nc.vector.tensor_mul(p2, l(1), d2)
                f2 = t3("f2")
                nc.vector.tensor_add(f2, p2, f4)
                d1 = t3("d1")
                nc.vector.tensor_sub(d1, f2, f3)
                p1 = t3("p1")
                nc.vector.tensor_mul(p1, l(0), d1)
                nc.vector.tensor_add(F3, p1, f3)

                nc.gpsimd.dma_start(y[s], F[:])

    import concourse.mybir as _mybir

    _split_multiwait_instructions(nc, _mybir)
    return nc


def _prep_inputs(x, Ws, bs):
    """Host-side shard + layout prep. Returns per-core input maps."""
    x = np.asarray(x, dtype=np.float32)
    Ws = np.asarray(Ws, dtype=np.float32)
    bs = np.asarray(bs, dtype=np.float32)

    W7 = np.zeros((64, 7), np.float32)
    b7 = np.zeros(7, np.float32)
    for j in range(7):
        W7[:, j] = Ws[j // 4, :, j % 4]
        b7[j] = bs[j // 4, j % 4]
    W2 = np.zeros((128, 16), np.float32)
    W2[0:64, 0:7] = W7
    W2[64:128, 8:15] = W7
    B2 = np.zeros((128, 1), np.float32)
    for g in range(4):
        for h in range(2):
            B2[32 * g + 8 * h:32 * g + 8 * h + 7, 0] = b7

    in_maps = []
    for c in range(N_CORES):
        xc = x[c * BC:(c + 1) * BC]
        xT2 = np.ascontiguousarray(
            xc.reshape(2, HALF, D).transpose(0, 2, 1).reshape(128, HALF)
        )
        in_maps.append({"xT2": xT2, "w2": W2, "b2": B2})
    return in_maps


def _gather_output(results):
    """Invert the device layout: yraw[s, 32g+r, 32u+2c+h] holds batch
    HALF*h + SUP_W*s + 2048*u + 512*g + 32*c + r (core-local)."""
    outs = []
    for c in range(N_CORES):
        yraw = np.asarray(results[c]["y"], dtype=np.float32).reshape(-1)
        yc = (
            yraw.reshape(NS, 4, 32, 4, 16, 2)
            .transpose(5, 0, 3, 1, 4, 2)
            .reshape(BC)
        )
        outs.append(yc)
    return np.concatenate(outs).astype(np.float32)


def run(inputs, trace=False, **run_kwargs):
    """Build, execute on 8 cores, and gather. Returns (y, BassKernelResults)."""
    from concourse.bass_utils import run_bass_kernel_spmd

    nc = build_program()
    in_maps = _prep_inputs(inputs["x"], inputs["Ws"], inputs["bs"])
    res = run_bass_kernel_spmd(
        nc, in_maps, core_ids=list(range(N_CORES)), trace=trace, **run_kwargs
    )
    return _gather_output(res.results), res


def kernel(x, Ws, bs):
    y, _ = run({"x": x, "Ws": Ws, "bs": bs})
    return y


if __name__ == "__main__":
    rng = np.random.default_rng(0)
    x = rng.standard_normal((B_TOTAL, D), dtype=np.float32)
    Ws = (rng.standard_normal((4, 64, 4)) * 0.1).astype(np.float32)
    bs = np.zeros((4, 4), np.float32)
    y = kernel(x, Ws, bs)
    print("kernel ran, y:", y.shape, y.dtype, y[:4])


# revision 3
# speedup vs baseline: 1.0079x; 1.0079x over previous
"""Trainium2 Bass kernel for nn_NetworksPlusCircuit.

Computation: y[b] = circuit(sigmoid(x[b] @ Ws + bs)) for x [1048576, 64].

Key simplification: the SDD circuit f(i) = pos_i*f(i+1) + neg_i*f(i+2) with
neg = 1-l collapses to f(i) == 1 for all i >= 8 (l + (1-l) = 1), so only
labelling columns 1..7 matter (literals 3 and 7 are categorical). The matmul
shrinks to [B,64] @ [64,7] and the circuit to a handful of elementwise ops:
    f7 = l7 + 1
    f6 = l6*l7 + 1
    f5 = l5*(f6-f7) + f7
    f4 = l4*(f5-f6) + f6
    f3 = l3*f4 + f5
    f2 = l2*(f3-f4) + f4
    f1 = l1*(f2-f3) + f3

Sharding: pure data parallel over batch across 8 cores. Per core the host
pre-transposes its x shard to xT2 [128, 65536] (two 64-row d-major halves
stacked on the partition dim) so the PE streams it as the moving operand
against a small stationary block-diagonal weight W2 [128, 16]. Matmul output
lands literal-on-partition in PSUM (two 512-wide chunks per 2-bank PSUM
tile); the scalar engine applies sigmoid (bias folded in as a per-partition
bias) while copying to SBUF, the DVE 32x32 block-transpose flips batch onto
partitions, and the circuit runs as wide strided elementwise ops. The host
inverts the layout permutation on the gathered outputs.

Per-core layout (core-local batch index):
    batch = 65536*h + 32768*t + 4096*v + 1024*g + 512*e + 32*c + r
  h: d-half (stacked on partitions 64*h+d), t: H-tile (2), v: psum-pair
  within H (8), g: partition group = matmul col-tile (4), e: psum free half
  (2), c: 32-block within 512 (16), r: batch-within-32 (32).
  Matmul chunk n = 8*p + 2*g + e (p = 8*t + v) covers w-cols [512n, 512n+512)
  of the half; after the 32x32 transpose, literal j of (h, e, c, r) sits at
  H[32g + r, 1024*v + 512*e + 32*c + 8*h + j].
  F[32g + r, 64*v + 32*e + 2*c + h] = f1;  stored flat to y[t].
"""

import sys

for _p in ("/opt/trn_rl_repo",):
    if _p not in sys.path:
        sys.path.insert(0, _p)

import numpy as np

N_CORES = 8
B_TOTAL = 1048576
D = 64
BC = B_TOTAL // N_CORES      # 131072 batch per core
HALF = BC // 2               # 65536 w-columns per half
NCH = HALF // 512            # 128 matmul chunks of 512 w-cols
NSUP = 8                     # X loads per core
SUP_W = HALF // NSUP         # 8192 w-cols per X load (4 MB)
NPAIR = 16                   # psum pair-tiles per core (8 chunks each)
NH = 2                       # H tiles per core (8 pairs each)


def _split_multiwait_instructions(nc, mybir):
    """This walrus build accepts at most one sync wait per instruction.
    Split any multi-wait instruction into single-wait NoOps on the same
    engine ahead of it (engines execute their queue in order, so semantics
    are unchanged)."""
    n_split = 0
    for fn in nc.m.functions:
        for blk in fn.blocks:
            insts = blk.instructions
            if not any(
                i.sync_info is not None and len(i.sync_info.on_wait) > 1
                for i in insts
            ):
                continue
            out = []
            for inst in insts:
                si = inst.sync_info
                if si is not None and len(si.on_wait) > 1:
                    waits = list(si.on_wait)
                    for k, w in enumerate(waits[:-1]):
                        nop = mybir.InstNoOp(
                            name=f"{inst.name}-sw{k}",
                            engine=inst.engine,
                            ins=[],
                            outs=[],
                            sync_info=mybir.SyncInfo(on_wait=[w], on_update=[]),
                        )
                        out.append(nop)
                        n_split += 1
                    inst.sync_info = mybir.SyncInfo(
                        on_wait=[waits[-1]], on_update=list(si.on_update)
                    )
                out.append(inst)
            blk.instructions = out
    return n_split


def build_program():
    import concourse.bass as bass
    import concourse.mybir as mybir
    from concourse import tile
    from contextlib import ExitStack

    f32 = mybir.dt.float32
    nc = bass.Bass("TRN2")
    xT2 = nc.dram_tensor("xT2", [128, HALF], f32, kind="ExternalInput")
    w2 = nc.dram_tensor("w2", [128, 16], f32, kind="ExternalInput")
    b2 = nc.dram_tensor("b2", [128, 1], f32, kind="ExternalInput")
    y = nc.dram_tensor("y", [NH, 128, 512], f32, kind="ExternalOutput")

    with tile.TileContext(nc) as tc:
        with ExitStack() as ctx:
            wpool = ctx.enter_context(tc.tile_pool(name="wpool", bufs=1))
            xpool = ctx.enter_context(tc.tile_pool(name="xpool", bufs=2))
            spool = ctx.enter_context(tc.tile_pool(name="spool", bufs=3))
            hpool = ctx.enter_context(tc.tile_pool(name="hpool", bufs=2))
            cpool = ctx.enter_context(tc.tile_pool(name="cpool", bufs=1))
            fpool = ctx.enter_context(tc.tile_pool(name="fpool", bufs=2))
            ppool = ctx.enter_context(
                tc.tile_pool(name="ppool", bufs=3, space="PSUM")
            )

            wt = wpool.tile([128, 16], f32)
            nc.sync.dma_start(wt[:], w2[:, :])
            bt = wpool.tile([128, 1], f32)
            nc.sync.dma_start(bt[:], b2[:, :])

            X = None
            for t in range(NH):
                H = hpool.tile([128, 8192], f32)
                for v in range(8):
                    p = 8 * t + v
                    if p % 2 == 0:
                        X = xpool.tile([128, SUP_W], f32, name="X", tag="X")
                        s = p // 2
                        nc.sync.dma_start(
                            X[:], xT2[:, s * SUP_W:(s + 1) * SUP_W]
                        )
                    ps = ppool.tile([128, 1024], f32)
                    for g in range(4):
                        for e in range(2):
                            n = 8 * p + 2 * g + e
                            xoff = 512 * (n % 16)
                            nc.tensor.matmul(
                                ps[32 * g:32 * g + 16, 512 * e:512 * (e + 1)],
                                wt[:, :],
                                X[:, xoff:xoff + 512],
                                start=True,
                                stop=True,
                                tile_position=(0, 32 * g),
                            )
                    S = spool.tile([128, 1024], f32)
                    nc.scalar.activation(
                        S[:],
                        ps[:],
                        mybir.ActivationFunctionType.Sigmoid,
                        bias=bt[:, 0:1],
                        scale=1.0,
                    )
                    nc.vector.transpose(H[:, 1024 * v:1024 * (v + 1)], S[:])

                # circuit over H [128, 8192]; literal j of (h-half) at free
                # slot 32*blk + 8*h + j
                H3 = H.rearrange("p (b q) -> p b q", q=32)
                l = lambda j: H3[:, :, j:16:8]  # noqa: E731

                def t3(name):
                    t_ = cpool.tile([128, 512], f32, name=name, tag=name)
                    return t_.rearrange("p (b q) -> p b q", q=2)

                F = fpool.tile([128, 512], f32)
                F3 = F.rearrange("p (b q) -> p b q", q=2)

                f7 = t3("f7")
                nc.vector.tensor_scalar_add(f7, l(6), 1.0)
                pr = t3("pr")
                nc.vector.tensor_mul(pr, l(5), l(6))
                f6 = t3("f6")
                nc.vector.tensor_scalar_add(f6, pr, 1.0)
                d = t3("d")
                nc.vector.tensor_sub(d, f6, f7)
                pr2 = t3("pr2")
                nc.vector.tensor_mul(pr2, l(4), d)
                f5 = t3("f5")
                nc.vector.tensor_add(f5, pr2, f7)
                d2 = t3("d2")
                nc.vector.tensor_sub(d2, f5, f6)
                pr3 = t3("pr3")
                nc.vector.tensor_mul(pr3, l(3), d2)
                f4 = t3("f4")
                nc.vector.tensor_add(f4, pr3, f6)
                pr4 = t3("pr4")
                nc.vector.tensor_mul(pr4, l(2), f4)
                f3 = t3("f3")
                nc.vector.tensor_add(f3, pr4, f5)
                d3 = t3("d3")
                nc.vector.tensor_sub(d3, f3, f4)
                pr5 = t3("pr5")
                nc.vector.tensor_mul(pr5, l(1), d3)
                f2 = t3("f2")
                nc.vector.tensor_add(f2, pr5, f4)
                d4 = t3("d4")
                nc.vector.tensor_sub(d4, f2, f3)
                pr6 = t3("pr6")
                nc.vector.tensor_mul(pr6, l(0), d4)
                nc.vector.tensor_add(F3, pr6, f3)

                nc.gpsimd.dma_start(y[t], F[:])

    import concourse.mybir as _mybir

    _split_multiwait_instructions(nc, _mybir)
    return nc


def _prep_inputs(x, Ws, bs):
    """Host-side shard + layout prep. Returns per-core input maps."""
    x = np.asarray(x, dtype=np.float32)
    Ws = np.asarray(Ws, dtype=np.float32)
    bs = np.asarray(bs, dtype=np.float32)

    W7 = np.zeros((64, 7), np.float32)
    b7 = np.zeros(7, np.float32)
    for j in range(7):
        W7[:, j] = Ws[j // 4, :, j % 4]
        b7[j] = bs[j // 4, j % 4]
    W2 = np.zeros((128, 16), np.float32)
    W2[0:64, 0:7] = W7
    W2[64:128, 8:15] = W7
    B2 = np.zeros((128, 1), np.float32)
    for g in range(4):
        for h in range(2):
            B2[32 * g + 8 * h:32 * g + 8 * h + 7, 0] = b7

    in_maps = []
    for c in range(N_CORES):
        xc = x[c * BC:(c + 1) * BC]
        xT2 = np.ascontiguousarray(
            xc.reshape(2, HALF, D).transpose(0, 2, 1).reshape(128, HALF)
        )
        in_maps.append({"xT2": xT2, "w2": W2, "b2": B2})
    return in_maps


def _gather_output(results):
    """Invert the device layout; see module docstring for the index map."""
    outs = []
    for c in range(N_CORES):
        yraw = np.asarray(results[c]["y"], dtype=np.float32).reshape(-1)
        yc = (
            yraw.reshape(NH, 4, 32, 8, 2, 16, 2)   # t g r v e c h
            .transpose(6, 0, 3, 1, 4, 5, 2)        # h t v g e c r
            .reshape(BC)
        )
        outs.append(yc)
    return np.concatenate(outs).astype(np.float32)


def run(inputs, trace=False, **run_kwargs):
    """Build, execute on 8 cores, and gather. Returns (y, BassKernelResults)."""
    from concourse.bass_utils import run_bass_kernel_spmd

    nc = build_program()
    in_maps = _prep_inputs(inputs["x"], inputs["Ws"], inputs["bs"])
    res = run_bass_kernel_spmd(
        nc, in_maps, core_ids=list(range(N_CORES)), trace=trace, **run_kwargs
    )
    return _gather_output(res.results), res


def kernel(x, Ws, bs):
    y, _ = run({"x": x, "Ws": Ws, "bs": bs})
    return y


if __name__ == "__main__":
    rng = np.random.default_rng(0)
    x = rng.standard_normal((B_TOTAL, D), dtype=np.float32)
    Ws = (rng.standard_normal((4, 64, 4)) * 0.1).astype(np.float32)
    bs = np.zeros((4, 4), np.float32)
    y = kernel(x, Ws, bs)
    print("kernel ran, y:", y.shape, y.dtype, y[:4])
